# revision 35
# baseline (speedup 1.0000x reference)
"""Multi-head self-attention (B=8, E=512, heads=8, S=1024) on 8 trn2 cores.

Sharding: data-parallel over batch — core b computes batch element b end to
end (no collectives). Weights replicated, pre-transposed on host.

v2 design (cost-model-driven; see git history for the v1 layout):
  - xsT ([C, S]) is built on the HOST (the reference's reshape is a pure
    memory reinterpretation, so xsT = x[b].reshape(S, C).T in numpy). This
    removes all on-device PE transposes and their ACT/DVE copy traffic.
  - Loop order n (query half) OUTER, hp (head pair) INNER. The output
    projection for half n runs as PE filler inside half n+1's attention, so
    only the last half's projection sits in the tail.
  - All "filler" matmul groups (v-proj, q/k-proj, out-proj) accumulate in a
    dedicated 1-bank PSUM tag ("fg") so they never perturb the scores
    double-buffer; fillers are placed at fixed (n, hp, t2) slots chosen so
    every operand arrives just-in-time.
  - PSUM budget (8 banks): scores [128,1024] x2 (4) + ctx [65,512] x3 (3) +
    fg [128,512] x1 (1).
  - Warmup: gpsimd memset + 16 dummy matmuls finish the PE p-state ramp
    (0.65->2.4 GHz) before real work; a dummy exp preloads the ACT table.
  - Host packs wq+wk into per-m-slice tensors and biases/ones into one misc
    tensor; input DMAs are ordered by first use (HWDGE serializes issues).
  - Softmax denominators ride as a 65th stationary column of v (ones), so
    ctx PSUM row 64 accumulates them for free. Normalization: reciprocal on
    DVE, partition-broadcast via a DRAM bounce (mid-kernel, latency hidden)
    or via a K=1 PE matmul against a ones row (final drain, latency-critical).
"""

import numpy as np
from contextlib import ExitStack

import concourse.bass as bass
import concourse.mybir as mybir
import concourse.tile as tile
from concourse import bacc
from concourse.bass_utils import run_bass_kernel_spmd

B = 8
C = 512
HH = 32
WW = 32
S = HH * WW            # 1024
HEADS = 8
HD = C // HEADS        # 64
CB = C // 128          # 4 channel blocks
TB = S // 128          # 8 token blocks
CHUNK = 512            # fp32 moving-operand max
NCH = S // CHUNK       # 2
F32 = mybir.dt.float32
MM_DT = mybir.dt.float32r  # full-rate PE at N>=256

EXP = mybir.ActivationFunctionType.Exp
ADD = mybir.AluOpType.add
MULT = mybir.AluOpType.mult

# misc tensor column layout
MC_BVBC = 0          # [0:512)   bv broadcast along free dim
MC_BIAS = 512        # [512:524) bq(4), bk(4), bo(4) per-chunk scalars
MC_ONES8 = 524       # [524:532) ones for v's denominator columns
MC_SEL = 532         # [532:660) 2x128 selector (rows 64/65) for the drain
                     #           pair-broadcast matmul
MISC_W = 660


def build_nc(reps=1):
    nc = bacc.Bacc()
    xst_d = [nc.declare_dram_parameter(f"xst{j}", [128, S], MM_DT, isOutput=False)
             for j in range(CB)]
    wqk_d = [nc.declare_dram_parameter(f"wqk{m}", [128, 2 * C], MM_DT, isOutput=False)
             for m in range(CB)]
    wv_d = [nc.declare_dram_parameter(f"wv{h}", [128, 2 * C], MM_DT, isOutput=False)
            for h in range(2)]
    wo_d = [nc.declare_dram_parameter(f"wo{h}", [128, 2 * C], MM_DT, isOutput=False)
            for h in range(2)]
    misc_d = nc.declare_dram_parameter("misc", [128, MISC_W], F32, isOutput=False)
    out_d = nc.declare_dram_parameter("out", [C, S], F32, isOutput=True)

    with tile.TileContext(nc) as tc, ExitStack() as ctx:
        pools = _make_pools(ctx, tc)
        for _ in range(reps):
            _emit(pools, nc, xst_d, wqk_d, wv_d, wo_d, misc_d, out_d)
    nc.compile()
    return nc


def _make_pools(ctx, tc):
    return {
        "sb": ctx.enter_context(tc.tile_pool(name="sb", bufs=1)),
        "ps": ctx.enter_context(tc.tile_pool(name="ps", bufs=2, space="PSUM")),
        "ep": ctx.enter_context(tc.tile_pool(name="ep", bufs=6)),
        "np": ctx.enter_context(tc.tile_pool(name="npool", bufs=6)),
        "dr": ctx.enter_context(tc.tile_pool(name="drpool", bufs=4, space="DRAM")),
    }


def _emit(pools, nc, xst_d, wqk_d, wv_d, wo_d, misc_d, out_d):
    sb = pools["sb"]
    ps = pools["ps"]
    ep = pools["ep"]
    np_pool = pools["np"]
    dr_pool = pools["dr"]

    def sc_tile():
        return ps.tile([128, 1024], F32, tag="sc", bufs=2, name="sc")

    def cx_tile():
        return ps.tile([65, 512], F32, tag="cx", bufs=3, name="cx")

    def fg_tile():
        return ps.tile([128, 512], F32, tag="fg", bufs=1, name="fg")

    # ---- input DMAs, ordered by first use (HWDGE serializes issues) ----
    wqk = [sb.tile([128, 2 * C], MM_DT, tag=f"wqk{m}", name=f"wqk{m}")
           for m in range(CB)]
    xsT = [sb.tile([128, S], MM_DT, tag=f"xsT{j}", name=f"xsT{j}") for j in range(CB)]
    wv = [sb.tile([128, 2 * C], MM_DT, tag=f"wv{h}", name=f"wv{h}") for h in range(2)]
    wo = [sb.tile([128, 2 * C], MM_DT, tag=f"wo{h}", name=f"wo{h}") for h in range(2)]
    misc = sb.tile([128, MISC_W], F32, tag="misc", name="misc")

    nc.sync.dma_start(wqk[0], wqk_d[0][:, :])
    for j in range(CB):
        nc.sync.dma_start(xsT[j][:, 0:512], xst_d[j][:, 0:512])
    nc.sync.dma_start(misc, misc_d[:, :])
    nc.sync.dma_start(wv[0], wv_d[0][:, :])
    nc.sync.dma_start(wv[1], wv_d[1][:, :])
    for j in range(CB):
        nc.sync.dma_start(xsT[j][:, 512:1024], xst_d[j][:, 512:1024])
    for m in range(1, CB):
        nc.sync.dma_start(wqk[m], wqk_d[m][:, :])
    nc.sync.dma_start(wo[0], wo_d[0][:, :])
    nc.sync.dma_start(wo[1], wo_d[1][:, :])

    def w_slice(kind, j, m):
        # stationary [c_in 128, c_out 128] for projection matmuls
        if kind == "q":
            return wqk[m][:, j * 256:j * 256 + 128]
        if kind == "k":
            return wqk[m][:, j * 256 + 128:(j + 1) * 256]
        if kind == "v":
            return wv[j // 2][:, (j % 2) * 512:(j % 2) * 512 + 512]  # moving, 512 wide
        if kind == "o":
            return wo[j // 2][:, (j % 2) * 512 + m * 128:(j % 2) * 512 + (m + 1) * 128]
        raise KeyError(kind)

    def bias_ap(name, m):
        off = {"bq": 0, "bk": 4, "bo": 8}[name]
        return misc[:, MC_BIAS + off + m:MC_BIAS + off + m + 1]

    # ---- warmup: finish PE p-state ramp + preload the Exp ACT table ----
    wt = sb.tile([128, 512], F32, tag="wt", name="wt")
    nc.gpsimd.memset(wt[:, :], 0.0)
    wte = sb.tile([128, 8], F32, tag="wte", name="wte")
    nc.scalar.activation(wte, wt[:, 0:8], EXP, scale=0.125)
    for i in range(20):
        pt = fg_tile() if i % 2 == 0 else sc_tile()
        nc.tensor.matmul(pt[:, 0:512], lhsT=wt[:, 0:128].bitcast(MM_DT),
                         rhs=wt[:, 0:512].bitcast(MM_DT),
                         start=True, stop=True)

    # ---- projection groups ----
    qT = [sb.tile([128, S], MM_DT, tag=f"qT{m}", name=f"qT{m}") for m in range(CB)]
    kT = [sb.tile([128, S], MM_DT, tag=f"kT{m}", name=f"kT{m}") for m in range(CB)]
    v = [sb.tile([128, HEADS * (HD + 1)], MM_DT, tag=f"v{i}", name=f"v{i}")
         for i in range(TB)]
    zT = [sb.tile([128, S], MM_DT, tag=f"zT{hp}", name=f"zT{hp}") for hp in range(CB)]
    outT = [sb.tile([128, S], F32, tag=f"outT{m}", name=f"outT{m}") for m in range(CB)]

    def qk_group(kind, m, n, bank=None):
        # qT/kT[m][:, n-half] = W[:, m-slice].T @ xsT[:, n-half] + bias
        dest = qT if kind == "q" else kT
        pt = bank() if bank else fg_tile()
        for j in range(CB):
            nc.tensor.matmul(
                pt[:, 0:512],
                lhsT=w_slice(kind, j, m),
                rhs=xsT[j][:, n * CHUNK:(n + 1) * CHUNK],
                start=(j == 0), stop=(j == CB - 1),
            )
        nc.vector.tensor_scalar_add(
            dest[m][:, n * CHUNK:(n + 1) * CHUNK], pt[:, 0:512],
            bias_ap("bq" if kind == "q" else "bk", m),
        )

    def v_group(i, bank=None):
        # v[i] token-major [128, 8*65]: head h dims at h*65..h*65+63, ones col
        # at h*65+64 (softmax denominator rides the ctx matmul).
        pt = bank() if bank else fg_tile()
        for j in range(CB):
            nc.tensor.matmul(
                pt[:, 0:512],
                lhsT=xsT[j][:, i * 128:(i + 1) * 128],
                rhs=w_slice("v", j, 0),
                start=(j == 0), stop=(j == CB - 1),
            )
        v3 = v[i].rearrange("p (h d) -> p h d", d=HD + 1)
        nc.vector.tensor_tensor(
            v3[:, :, 0:HD],
            pt[:, 0:512].rearrange("p (h d) -> p h d", d=HD),
            misc[:, MC_BVBC:MC_BVBC + 512].rearrange("p (h d) -> p h d", d=HD),
            ADD,
        )
        nc.vector.tensor_copy(v3[:, :, HD], misc[:, MC_ONES8:MC_ONES8 + 8])

    held = {}

    def out_mm(pt, m, n, j):
        nc.tensor.matmul(
            pt[:, 0:512],
            lhsT=w_slice("o", j, m),
            rhs=zT[j][:, n * CHUNK:(n + 1) * CHUNK],
            start=(j == 0), stop=(j == CB - 1),
        )

    def out_emit(pt, m, n, split=1, eng="v"):
        # bias + store for a finished out-proj accumulation; eng="a" runs the
        # bias on the ACT engine (idle in the tail, where DVE serializes)
        w = 512 // split
        for s in range(split):
            lo, hi = s * w, (s + 1) * w
            dst = outT[m][:, n * CHUNK + lo:n * CHUNK + hi]
            if eng == "a":
                nc.scalar.activation(
                    dst, pt[:, lo:hi], mybir.ActivationFunctionType.Identity,
                    bias=bias_ap("bo", m),
                )
            else:
                nc.vector.tensor_scalar_add(dst, pt[:, lo:hi], bias_ap("bo", m))
            nc.sync.dma_start(
                out_d[m * 128:(m + 1) * 128, n * CHUNK + lo:n * CHUNK + hi],
                dst,
            )

    def out_group(m, n):
        # outT[m][:, n-half] = Wo[m-slice].T @ zT[:, n-half] + bo, then DMA
        pt = fg_tile()
        for j in range(CB):
            out_mm(pt, m, n, j)
        out_emit(pt, m, n)

    def cx_half():
        return ps.tile([128, 512], F32, tag="cx", bufs=3, name="cx")

    def out_partial(m, n, j, bank=None):
        # incremental out-proj chunk into a held accumulation (tail prep)
        if (m, n) not in held:
            held[(m, n)] = (bank or fg_tile)()
        out_mm(held[(m, n)], m, n, j)

    def out_finish(m, n, split=1, eng="v"):
        pt = held.pop((m, n))
        out_mm(pt, m, n, CB - 1)
        out_emit(pt, m, n, split=split, eng=eng)

    def sc_half():
        return ps.tile([128, 512], F32, tag="sc", bufs=2, name="sc")

    # ---- upfront groups (operands arrive via the first DMAs); spread over
    # sc + fg banks so they don't serialize on one accumulator. v0-v3 run
    # while the PE is otherwise DMA-idle; k01 is deferred (needed at t2=4). --
    qk_group("k", 0, 0)                 # fg
    qk_group("q", 0, 0, bank=sc_half)   # sc slot A

    # filler schedule: (n, hp) -> {t2: thunk}; chosen so every group lands
    # just before its first consumer, consecutive fg users sit >= 1 iteration
    # apart (the fg matmul+bias round-trip is ~1us), and the tail carries no
    # q/k/v work.
    filler = {}

    def put(n, hp, t2, fn, *a, **k):
        filler.setdefault((n, hp), {}).setdefault(t2, []).append((fn, a, k))

    put(0, 0, 0, qk_group, "k", 0, 1)
    put(0, 0, 1, v_group, 4)
    put(0, 0, 2, v_group, 5)
    put(0, 0, 3, qk_group, "k", 1, 0)
    put(0, 0, 4, v_group, 6)
    put(0, 0, 5, qk_group, "q", 1, 0)
    put(0, 0, 6, v_group, 7)
    put(0, 0, 7, qk_group, "k", 1, 1)
    for mm in range(2, CB):
        put(0, mm - 1, 1, qk_group, "k", mm, 0)
        put(0, mm - 1, 3, qk_group, "q", mm, 0)
        put(0, mm - 1, 5, qk_group, "k", mm, 1)
    put(0, 3, 1, qk_group, "q", 0, 1)
    put(0, 3, 3, qk_group, "q", 1, 1)
    put(0, 3, 5, qk_group, "q", 2, 1)
    put(1, 0, 1, qk_group, "q", 3, 1)
    put(1, 1, 1, out_group, 0, 0)
    put(1, 1, 3, out_group, 1, 0)
    put(1, 1, 5, out_group, 2, 0)
    put(1, 2, 1, out_group, 3, 0)
    # tail prep: accumulate out(m=0/1, n=1) over already-drained zT chunks.
    # m0 in fg (free of drain(1,2)'s broadcast by t2=3); m1 in the cx slot
    # vacated by drain(1,2)'s accumulators. The j=2 chunks are saved for the
    # tail itself (PE-warming filler inside the final drain's bubble).
    put(1, 3, 3, out_partial, 0, 1, 0)
    put(1, 3, 4, out_partial, 1, 1, 0, bank=cx_half)
    put(1, 3, 5, out_partial, 0, 1, 1)
    put(1, 3, 5, out_partial, 1, 1, 1)

    def drain_pair(cps, hp, n, bank, hop):
        # Normalize both ctx accumulators of a head pair: reciprocals of the
        # two denominator rows (psum row 64 of each), ONE K=2 matmul against
        # a 2x128 selector broadcasts recipA to partitions 0-63 and recipB to
        # 64-127, one PSUM->SBUF hop (DVE can't read two PSUM operands), two
        # multiplies into zT. Single PE instruction -> no boundary PE stall;
        # ~2.5us total so the cx rotation (bufs=3) never blocks.
        rs = [np_pool.tile([65, 512], F32, tag="rs", name="rs") for _ in range(2)]
        for half in range(2):
            nc.vector.reciprocal(rs[half][64:65, :], cps[half][64:65, :])
        rb = bank()
        for half in range(2):
            # K=1 broadcast matmuls into the two column-tiles of one bank:
            # back-to-back on PE, single PSUM->SBUF hop afterwards
            nc.tensor.matmul(
                rb[half * 64:(half + 1) * 64, 0:512],
                lhsT=misc[64:65, MC_SEL:MC_SEL + 64],
                rhs=rs[half][64:65, :],
                start=True, stop=True, tile_position=(64, half * 64),
            )
        rbs = np_pool.tile([128, 512], F32, tag="rbs", name="rbs")
        hop(rbs, rb[:, 0:512])
        for half in range(2):
            nc.vector.tensor_tensor(
                zT[hp][half * 64:(half + 1) * 64, n * CHUNK:(n + 1) * CHUNK],
                cps[half][0:64, :], rbs[half * 64:(half + 1) * 64, :], MULT,
            )

    # ---- attention: n outer, hp inner; scores/exp emitted one t2 ahead so
    # a ctx matmul waiting on exp never blocks the scores pipeline ----
    for n in range(NCH):
        for hp in range(CB):
            qh, kh = qT[hp], kT[hp]
            fills = filler.get((n, hp), {})
            cps = [cx_tile(), cx_tile()]   # head A, head B
            Es = [None] * TB

            def emit_se(t2):
                sc = sc_tile()
                nc.tensor.matmul(
                    sc[:, 0:512],
                    lhsT=kh[0:64, t2 * 128:(t2 + 1) * 128],
                    rhs=qh[0:64, n * CHUNK:(n + 1) * CHUNK],
                    start=True, stop=True,
                    tile_position=(0, 0),
                )
                nc.tensor.matmul(
                    sc[:, 512:1024],
                    lhsT=kh[64:128, t2 * 128:(t2 + 1) * 128],
                    rhs=qh[64:128, n * CHUNK:(n + 1) * CHUNK],
                    start=True, stop=True,
                    tile_position=(64, 0),
                )
                Es[t2] = ep.tile([128, 1024], MM_DT, tag="E", name="E")
                nc.scalar.activation(Es[t2], sc, EXP, scale=1.0 / np.sqrt(HD))

            # two-ahead: se(t2+2) is emitted before ctx(t2) so the ACT
            # pipeline never waits on a ctx-blocked PE (the dependency cycle
            # exp(t2)->ctx(t2)->scores(t2+2)->exp(t2+2) would be 1.11us,
            # longer than one 1.04us exp)
            emit_se(0)
            emit_se(1)
            if n == 0 and hp == 0:
                # v0-v3 land in the startup DMA window; two lanes (fg + the
                # one spare cx slot) so they don't serialize on one bank
                v_group(0)
                v_group(1, bank=cx_half)
                v_group(2)
                v_group(3)
            for t2 in range(TB):
                if t2 + 2 < TB:
                    emit_se(t2 + 2)
                for half in range(2):
                    h = 2 * hp + half
                    nc.tensor.matmul(
                        cps[half][0:HD + 1, :],
                        lhsT=v[t2][:, h * (HD + 1):(h + 1) * (HD + 1)],
                        rhs=Es[t2][:, half * 512:(half + 1) * 512],
                        start=(t2 == 0), stop=(t2 == TB - 1),
                    )
                for fn, a, kw in fills.get(t2, []):
                    fn(*a, **kw)

            if (n == NCH - 1) and (hp == CB - 1):
                # tail: m2's independent chunks first (they keep the PE busy
                # under the drain's reciprocal latency), then the final drain
                # on the freed scores banks with the hop on tail-idle ACT
                for j in range(CB - 1):
                    out_partial(2, n, j, bank=sc_half)
                drain_pair(cps, hp, n, sc_half, nc.scalar.copy)
                out_partial(0, n, 2)
                out_partial(1, n, 2)
            else:
                drain_pair(cps, hp, n, fg_tile, nc.vector.tensor_copy)

    # ---- tail: m3's independent chunks (cx slot freed by the drain), then
    # finish all four held accumulations; biases alternate DVE/ACT so neither
    # engine serializes the four stores; the last store is split so its first
    # DMA overlaps the second half's bias-add ----
    nl = NCH - 1
    for j in range(CB - 1):
        out_partial(3, nl, j, bank=cx_half)
    out_finish(0, nl, eng="v")
    out_finish(1, nl, eng="a")
    out_finish(2, nl, eng="v")
    out_finish(3, nl, split=2, eng="a")


_NC_CACHE = None


def _get_nc():
    global _NC_CACHE
    if _NC_CACHE is None:
        _NC_CACHE = build_nc()
    return _NC_CACHE


def _in_maps(x, Wq, bq, Wk, bk, Wv, bv, Wo, bo):
    x = np.ascontiguousarray(np.asarray(x, np.float32))
    wqT = np.asarray(Wq, np.float32).T   # [c_in, c_out]
    wkT = np.asarray(Wk, np.float32).T
    wvT = np.asarray(Wv, np.float32).T
    woT = np.asarray(Wo, np.float32).T

    base = {}
    # wqk{m}: [128, (j, q|k, 128)] — stationary slices for qk_group
    for m in range(CB):
        t = np.empty((128, 2 * C), np.float32)
        for j in range(CB):
            t[:, j * 256:j * 256 + 128] = wqT[j * 128:(j + 1) * 128,
                                              m * 128:(m + 1) * 128]
            t[:, j * 256 + 128:(j + 1) * 256] = wkT[j * 128:(j + 1) * 128,
                                                    m * 128:(m + 1) * 128]
        base[f"wqk{m}"] = t
    for h in range(2):
        base[f"wv{h}"] = np.ascontiguousarray(
            np.concatenate([wvT[(2 * h) * 128:(2 * h + 1) * 128, :],
                            wvT[(2 * h + 1) * 128:(2 * h + 2) * 128, :]], axis=1))
        base[f"wo{h}"] = np.ascontiguousarray(
            np.concatenate([woT[(2 * h) * 128:(2 * h + 1) * 128, :],
                            woT[(2 * h + 1) * 128:(2 * h + 2) * 128, :]], axis=1))
    mi = np.zeros((128, MISC_W), np.float32)
    mi[:, MC_BVBC:MC_BVBC + 512] = np.asarray(bv, np.float32)[None, :]
    for j in range(CB):
        mi[:, MC_BIAS + j] = np.asarray(bq, np.float32)[j * 128:(j + 1) * 128]
        mi[:, MC_BIAS + 4 + j] = np.asarray(bk, np.float32)[j * 128:(j + 1) * 128]
        mi[:, MC_BIAS + 8 + j] = np.asarray(bo, np.float32)[j * 128:(j + 1) * 128]
    mi[:, MC_ONES8:MC_ONES8 + 8] = 1.0
    mi[64, MC_SEL:MC_SEL + 64] = 1.0       # ones row for the drain broadcast
    base["misc"] = mi

    maps = []
    for b in range(B):
        xsT = np.ascontiguousarray(x[b].reshape(S, C).T)  # [C, S]
        m = dict(base)
        for j in range(CB):
            m[f"xst{j}"] = np.ascontiguousarray(xsT[j * 128:(j + 1) * 128, :])
        maps.append(m)
    return maps


def _run(trace=False, **inputs):
    nc = _get_nc()
    maps = _in_maps(**inputs)
    res = run_bass_kernel_spmd(nc, maps, core_ids=list(range(B)), trace=trace)
    out = np.stack(
        [np.asarray(res.results[b]["out"]).reshape(C, HH, WW) for b in range(B)]
    ).astype(np.float32)
    return out, res


def kernel(**inputs):
    out, _ = _run(trace=False, **inputs)
    return out


# revision 43
# speedup vs baseline: 1.0646x; 1.0646x over previous
"""Multi-head self-attention (B=8, E=512, heads=8, S=1024) on 8 trn2 cores.

Sharding: data-parallel over batch — core b computes batch element b end to
end (no collectives). Weights replicated, pre-transposed on host.

v2 design (cost-model-driven; see git history for the v1 layout):
  - xsT ([C, S]) is built on the HOST (the reference's reshape is a pure
    memory reinterpretation, so xsT = x[b].reshape(S, C).T in numpy). This
    removes all on-device PE transposes and their ACT/DVE copy traffic.
  - Loop order n (query half) OUTER, hp (head pair) INNER. The output
    projection for half n runs as PE filler inside half n+1's attention, so
    only the last half's projection sits in the tail.
  - All "filler" matmul groups (v-proj, q/k-proj, out-proj) accumulate in a
    dedicated 1-bank PSUM tag ("fg") so they never perturb the scores
    double-buffer; fillers are placed at fixed (n, hp, t2) slots chosen so
    every operand arrives just-in-time.
  - PSUM budget (8 banks): scores [128,1024] x2 (4) + ctx [65,512] x3 (3) +
    fg [128,512] x1 (1).
  - Warmup: gpsimd memset + 16 dummy matmuls finish the PE p-state ramp
    (0.65->2.4 GHz) before real work; a dummy exp preloads the ACT table.
  - Host packs wq+wk into per-m-slice tensors and biases/ones into one misc
    tensor; input DMAs are ordered by first use (HWDGE serializes issues).
  - Softmax denominators ride as a 65th stationary column of v (ones), so
    ctx PSUM row 64 accumulates them for free. Normalization: reciprocal on
    DVE, partition-broadcast via a DRAM bounce (mid-kernel, latency hidden)
    or via a K=1 PE matmul against a ones row (final drain, latency-critical).
"""

import numpy as np
from contextlib import ExitStack

import concourse.bass as bass
import concourse.mybir as mybir
import concourse.tile as tile
from concourse import bacc
from concourse.bass_utils import run_bass_kernel_spmd

B = 8
C = 512
HH = 32
WW = 32
S = HH * WW            # 1024
HEADS = 8
HD = C // HEADS        # 64
CB = C // 128          # 4 channel blocks
TB = S // 128          # 8 token blocks
CHUNK = 512            # fp32 moving-operand max
NCH = S // CHUNK       # 2
F32 = mybir.dt.float32
MM_DT = mybir.dt.float32r  # full-rate PE at N>=256

EXP = mybir.ActivationFunctionType.Exp
ADD = mybir.AluOpType.add
MULT = mybir.AluOpType.mult

# misc tensor column layout
MC_BVBC = 0          # [0:512)   bv broadcast along free dim
MC_BIAS = 512        # [512:524) bq(4), bk(4), bo(4) per-chunk scalars
MC_ONES8 = 524       # [524:532) ones for v's denominator columns
MC_SEL = 532         # [532:660) 2x128 selector (rows 64/65) for the drain
                     #           pair-broadcast matmul
MISC_W = 660


def build_nc(reps=1):
    nc = bacc.Bacc()
    xst_d = [nc.declare_dram_parameter(f"xst{j}", [128, S], MM_DT, isOutput=False)
             for j in range(CB)]
    wqk_d = [nc.declare_dram_parameter(f"wqk{m}", [128, 2 * C], MM_DT, isOutput=False)
             for m in range(CB)]
    wv_d = [nc.declare_dram_parameter(f"wv{h}", [128, 2 * C], MM_DT, isOutput=False)
            for h in range(2)]
    wo_d = [nc.declare_dram_parameter(f"wo{h}", [128, 2 * C], MM_DT, isOutput=False)
            for h in range(2)]
    misc_d = nc.declare_dram_parameter("misc", [128, MISC_W], F32, isOutput=False)
    out_d = nc.declare_dram_parameter("out", [C, S], F32, isOutput=True)

    with tile.TileContext(nc) as tc, ExitStack() as ctx:
        pools = _make_pools(ctx, tc)
        for _ in range(reps):
            _emit(pools, nc, xst_d, wqk_d, wv_d, wo_d, misc_d, out_d)
    nc.compile()
    return nc


def _make_pools(ctx, tc):
    return {
        "sb": ctx.enter_context(tc.tile_pool(name="sb", bufs=1)),
        "ps": ctx.enter_context(tc.tile_pool(name="ps", bufs=2, space="PSUM")),
        "ep": ctx.enter_context(tc.tile_pool(name="ep", bufs=6)),
        "np": ctx.enter_context(tc.tile_pool(name="npool", bufs=6)),
        "dr": ctx.enter_context(tc.tile_pool(name="drpool", bufs=4, space="DRAM")),
    }


def _emit(pools, nc, xst_d, wqk_d, wv_d, wo_d, misc_d, out_d):
    sb = pools["sb"]
    ps = pools["ps"]
    ep = pools["ep"]
    np_pool = pools["np"]
    dr_pool = pools["dr"]

    def sc_tile():
        return ps.tile([128, 1024], F32, tag="sc", bufs=2, name="sc")

    def cx_tile():
        return ps.tile([65, 512], F32, tag="cx", bufs=3, name="cx")

    def fg_tile():
        return ps.tile([128, 512], F32, tag="fg", bufs=1, name="fg")

    # ---- input DMAs, ordered by first use (HWDGE serializes issues) ----
    wqk = [sb.tile([128, 2 * C], MM_DT, tag=f"wqk{m}", name=f"wqk{m}")
           for m in range(CB)]
    xsT = [sb.tile([128, S], MM_DT, tag=f"xsT{j}", name=f"xsT{j}") for j in range(CB)]
    wv = [sb.tile([128, 2 * C], MM_DT, tag=f"wv{h}", name=f"wv{h}") for h in range(2)]
    wo = [sb.tile([128, 2 * C], MM_DT, tag=f"wo{h}", name=f"wo{h}") for h in range(2)]
    misc = sb.tile([128, MISC_W], F32, tag="misc", name="misc")

    nc.sync.dma_start(wqk[0], wqk_d[0][:, :])
    for j in range(CB):
        nc.sync.dma_start(xsT[j][:, 0:512], xst_d[j][:, 0:512])
    nc.sync.dma_start(misc, misc_d[:, :])
    nc.sync.dma_start(wv[0], wv_d[0][:, :])
    nc.sync.dma_start(wv[1], wv_d[1][:, :])
    for j in range(CB):
        nc.sync.dma_start(xsT[j][:, 512:1024], xst_d[j][:, 512:1024])
    for m in range(1, CB):
        nc.sync.dma_start(wqk[m], wqk_d[m][:, :])
    nc.sync.dma_start(wo[0], wo_d[0][:, :])
    nc.sync.dma_start(wo[1], wo_d[1][:, :])

    def w_slice(kind, j, m):
        # stationary [c_in 128, c_out 128] for projection matmuls
        if kind == "q":
            return wqk[m][:, j * 256:j * 256 + 128]
        if kind == "k":
            return wqk[m][:, j * 256 + 128:(j + 1) * 256]
        if kind == "v":
            return wv[j // 2][:, (j % 2) * 512:(j % 2) * 512 + 512]  # moving, 512 wide
        if kind == "o":
            return wo[j // 2][:, (j % 2) * 512 + m * 128:(j % 2) * 512 + (m + 1) * 128]
        raise KeyError(kind)

    def bias_ap(name, m):
        off = {"bq": 0, "bk": 4, "bo": 8}[name]
        return misc[:, MC_BIAS + off + m:MC_BIAS + off + m + 1]

    # ---- warmup: finish PE p-state ramp + preload the Exp ACT table ----
    wt = sb.tile([128, 512], F32, tag="wt", name="wt")
    nc.gpsimd.memset(wt[:, :], 0.0)
    wte = sb.tile([128, 8], F32, tag="wte", name="wte")
    nc.scalar.activation(wte, wt[:, 0:8], EXP, scale=0.125)

    def warm_mm():
        # dependency-free matmul: finishes the PE p-state ramp / bridges
        # DMA-arrival gaps so pe_busy_start never resets (result unread)
        pt = sc_tile()
        nc.tensor.matmul(pt[:, 0:512], lhsT=wt[:, 0:128].bitcast(MM_DT),
                         rhs=wt[:, 0:512].bitcast(MM_DT),
                         start=True, stop=True)

    for i in range(8):
        warm_mm()

    # ---- projection groups ----
    qT = [sb.tile([128, S], MM_DT, tag=f"qT{m}", name=f"qT{m}") for m in range(CB)]
    kT = [sb.tile([128, S], MM_DT, tag=f"kT{m}", name=f"kT{m}") for m in range(CB)]
    v = [sb.tile([128, HEADS * (HD + 1)], MM_DT, tag=f"v{i}", name=f"v{i}")
         for i in range(TB)]
    zT = [sb.tile([128, S], MM_DT, tag=f"zT{hp}", name=f"zT{hp}") for hp in range(CB)]
    outT = [sb.tile([128, S], F32, tag=f"outT{m}", name=f"outT{m}") for m in range(CB)]

    def qk_group(kind, m, n, bank=None):
        # qT/kT[m][:, n-half] = W[:, m-slice].T @ xsT[:, n-half] + bias
        dest = qT if kind == "q" else kT
        pt = bank() if bank else fg_tile()
        for j in range(CB):
            nc.tensor.matmul(
                pt[:, 0:512],
                lhsT=w_slice(kind, j, m),
                rhs=xsT[j][:, n * CHUNK:(n + 1) * CHUNK],
                start=(j == 0), stop=(j == CB - 1),
            )
        nc.vector.tensor_scalar_add(
            dest[m][:, n * CHUNK:(n + 1) * CHUNK], pt[:, 0:512],
            bias_ap("bq" if kind == "q" else "bk", m),
        )

    def v_group(i, bank=None):
        # v[i] token-major [128, 8*65]: head h dims at h*65..h*65+63, ones col
        # at h*65+64 (softmax denominator rides the ctx matmul).
        pt = bank() if bank else fg_tile()
        for j in range(CB):
            nc.tensor.matmul(
                pt[:, 0:512],
                lhsT=xsT[j][:, i * 128:(i + 1) * 128],
                rhs=w_slice("v", j, 0),
                start=(j == 0), stop=(j == CB - 1),
            )
        v3 = v[i].rearrange("p (h d) -> p h d", d=HD + 1)
        nc.vector.tensor_tensor(
            v3[:, :, 0:HD],
            pt[:, 0:512].rearrange("p (h d) -> p h d", d=HD),
            misc[:, MC_BVBC:MC_BVBC + 512].rearrange("p (h d) -> p h d", d=HD),
            ADD,
        )
        nc.vector.tensor_copy(v3[:, :, HD], misc[:, MC_ONES8:MC_ONES8 + 8])

    held = {}

    def out_mm(pt, m, n, j):
        nc.tensor.matmul(
            pt[:, 0:512],
            lhsT=w_slice("o", j, m),
            rhs=zT[j][:, n * CHUNK:(n + 1) * CHUNK],
            start=(j == 0), stop=(j == CB - 1),
        )

    def out_emit(pt, m, n, split=1, eng="v", dma=None):
        # bias + store for a finished out-proj accumulation; eng="a" runs the
        # bias on the ACT engine and dma= picks the issuing sequencer (the
        # tail's stores otherwise serialize on SP's 650ns/DMA issue rate)
        w = 512 // split
        for s in range(split):
            lo, hi = s * w, (s + 1) * w
            dst = outT[m][:, n * CHUNK + lo:n * CHUNK + hi]
            if eng == "a":
                nc.scalar.activation(
                    dst, pt[:, lo:hi], mybir.ActivationFunctionType.Identity,
                    bias=bias_ap("bo", m),
                )
            else:
                nc.vector.tensor_scalar_add(dst, pt[:, lo:hi], bias_ap("bo", m))
            (dma or nc.sync).dma_start(
                out_d[m * 128:(m + 1) * 128, n * CHUNK + lo:n * CHUNK + hi],
                dst,
            )

    def out_group(m, n):
        # outT[m][:, n-half] = Wo[m-slice].T @ zT[:, n-half] + bo, then DMA
        pt = fg_tile()
        for j in range(CB):
            out_mm(pt, m, n, j)
        out_emit(pt, m, n)

    def cx_half():
        return ps.tile([128, 512], F32, tag="cx", bufs=3, name="cx")

    def out_partial(m, n, j, bank=None):
        # incremental out-proj chunk into a held accumulation (tail prep)
        if (m, n) not in held:
            held[(m, n)] = (bank or fg_tile)()
        out_mm(held[(m, n)], m, n, j)

    def out_finish(m, n, split=1, eng="v", dma=None):
        pt = held.pop((m, n))
        out_mm(pt, m, n, CB - 1)
        out_emit(pt, m, n, split=split, eng=eng, dma=dma)

    def sc_half():
        return ps.tile([128, 512], F32, tag="sc", bufs=2, name="sc")

    # ---- upfront groups (operands arrive via the first DMAs); spread over
    # sc + fg banks so they don't serialize on one accumulator. v0-v3 run
    # while the PE is otherwise DMA-idle; k01 is deferred (needed at t2=4).
    # k00's matmuls are paced by the xsT chunk DMAs, so warm matmuls ride
    # between them to keep the p-state ramp alive. ----
    ptk = fg_tile()
    for j in range(CB):
        nc.tensor.matmul(
            ptk[:, 0:512], lhsT=w_slice("k", j, 0), rhs=xsT[j][:, 0:CHUNK],
            start=(j == 0), stop=(j == CB - 1),
        )
        warm_mm()
        warm_mm()
    nc.vector.tensor_scalar_add(kT[0][:, 0:CHUNK], ptk[:, 0:512],
                                bias_ap("bk", 0))
    qk_group("q", 0, 0, bank=sc_half)   # sc slot A

    # filler schedule: (n, hp) -> {t2: thunk}; chosen so every group lands
    # just before its first consumer, consecutive fg users sit >= 1 iteration
    # apart (the fg matmul+bias round-trip is ~1us), and the tail carries no
    # q/k/v work.
    filler = {}

    def put(n, hp, t2, fn, *a, **k):
        filler.setdefault((n, hp), {}).setdefault(t2, []).append((fn, a, k))

    put(0, 0, 0, qk_group, "k", 0, 1)
    put(0, 0, 1, v_group, 4)
    put(0, 0, 2, v_group, 5)
    put(0, 0, 3, qk_group, "k", 1, 0)
    put(0, 0, 4, v_group, 6)
    put(0, 0, 5, qk_group, "q", 1, 0)
    put(0, 0, 6, v_group, 7)
    put(0, 0, 7, qk_group, "k", 1, 1)
    for mm in range(2, CB):
        put(0, mm - 1, 1, qk_group, "k", mm, 0)
        put(0, mm - 1, 3, qk_group, "q", mm, 0)
        put(0, mm - 1, 5, qk_group, "k", mm, 1)
    put(0, 3, 1, qk_group, "q", 0, 1)
    put(0, 3, 3, qk_group, "q", 1, 1)
    put(0, 3, 5, qk_group, "q", 2, 1)
    put(1, 0, 1, qk_group, "q", 3, 1)
    put(1, 1, 1, out_group, 0, 0)
    put(1, 1, 3, out_group, 1, 0)
    put(1, 1, 5, out_group, 2, 0)
    put(1, 2, 1, out_group, 3, 0)
    # tail prep: accumulate out(m=0/1, n=1) over already-drained zT chunks.
    # m0 in fg (free of drain(1,2)'s broadcast by t2=3); m1 in the cx slot
    # vacated by drain(1,2)'s accumulators. The j=2 chunks are saved for the
    # tail itself (PE-warming filler inside the final drain's bubble).
    put(1, 3, 3, out_partial, 0, 1, 0)
    put(1, 3, 4, out_partial, 1, 1, 0, bank=cx_half)
    put(1, 3, 5, out_partial, 0, 1, 1)
    put(1, 3, 5, out_partial, 1, 1, 1)

    def drain_bounce(cp, hp, half, n):
        # Mid-kernel normalization: 1/denominator at psum row 64, partition
        # broadcast via a DRAM bounce (no PE cost; ~6us latency hidden by the
        # cx rotation), multiply into zT.
        rs = np_pool.tile([65, 512], F32, tag="rs", name="rs")
        nc.vector.reciprocal(rs[64:65, :], cp[64:65, :])
        r_dram = dr_pool.tile([1, 512], F32, tag="r_dram", name="r_dram")
        nc.sync.dma_start(r_dram, rs[64:65, :])
        rb = np_pool.tile([64, 512], F32, tag="rb", name="rb")
        nc.sync.dma_start(rb, r_dram[0:1, :].partition_broadcast(64))
        nc.vector.tensor_tensor(
            zT[hp][half * 64:(half + 1) * 64, n * CHUNK:(n + 1) * CHUNK],
            cp[0:64, :], rb, MULT,
        )

    def drain_pair(cps, hp, n, bank, hop):
        # Normalize both ctx accumulators of a head pair: reciprocals of the
        # two denominator rows (psum row 64 of each), ONE K=2 matmul against
        # a 2x128 selector broadcasts recipA to partitions 0-63 and recipB to
        # 64-127, one PSUM->SBUF hop (DVE can't read two PSUM operands), two
        # multiplies into zT. Single PE instruction -> no boundary PE stall;
        # ~2.5us total so the cx rotation (bufs=3) never blocks.
        rs = [np_pool.tile([65, 512], F32, tag="rs", name="rs") for _ in range(2)]
        for half in range(2):
            nc.vector.reciprocal(rs[half][64:65, :], cps[half][64:65, :])
        rb = bank()
        for half in range(2):
            # K=1 broadcast matmuls into the two column-tiles of one bank:
            # back-to-back on PE, single PSUM->SBUF hop afterwards
            nc.tensor.matmul(
                rb[half * 64:(half + 1) * 64, 0:512],
                lhsT=misc[64:65, MC_SEL:MC_SEL + 64],
                rhs=rs[half][64:65, :],
                start=True, stop=True, tile_position=(64, half * 64),
            )
        rbs = np_pool.tile([128, 512], F32, tag="rbs", name="rbs")
        hop(rbs, rb[:, 0:512])
        for half in range(2):
            nc.vector.tensor_tensor(
                zT[hp][half * 64:(half + 1) * 64, n * CHUNK:(n + 1) * CHUNK],
                cps[half][0:64, :], rbs[half * 64:(half + 1) * 64, :], MULT,
            )

    # ---- attention: n outer, hp inner; scores/exp emitted one t2 ahead so
    # a ctx matmul waiting on exp never blocks the scores pipeline ----
    for n in range(NCH):
        for hp in range(CB):
            qh, kh = qT[hp], kT[hp]
            fills = filler.get((n, hp), {})
            cps = [cx_tile(), cx_tile()]   # head A, head B
            Es = [None] * TB

            def emit_se(t2):
                sc = sc_tile()
                nc.tensor.matmul(
                    sc[:, 0:512],
                    lhsT=kh[0:64, t2 * 128:(t2 + 1) * 128],
                    rhs=qh[0:64, n * CHUNK:(n + 1) * CHUNK],
                    start=True, stop=True,
                    tile_position=(0, 0),
                )
                nc.tensor.matmul(
                    sc[:, 512:1024],
                    lhsT=kh[64:128, t2 * 128:(t2 + 1) * 128],
                    rhs=qh[64:128, n * CHUNK:(n + 1) * CHUNK],
                    start=True, stop=True,
                    tile_position=(64, 0),
                )
                Es[t2] = ep.tile([128, 1024], MM_DT, tag="E", name="E")
                nc.scalar.activation(Es[t2], sc, EXP, scale=1.0 / np.sqrt(HD))

            # two-ahead: se(t2+2) is emitted before ctx(t2) so the ACT
            # pipeline never waits on a ctx-blocked PE (the dependency cycle
            # exp(t2)->ctx(t2)->scores(t2+2)->exp(t2+2) would be 1.11us,
            # longer than one 1.04us exp)
            emit_se(0)
            emit_se(1)
            if n == 0 and hp == 0:
                # v0-v3 land in the startup DMA window; two lanes (fg + the
                # one spare cx slot) so they don't serialize on one bank
                v_group(0)
                v_group(1, bank=cx_half)
                v_group(2)
                v_group(3)
            for t2 in range(TB):
                if t2 + 2 < TB:
                    emit_se(t2 + 2)
                for half in range(2):
                    h = 2 * hp + half
                    nc.tensor.matmul(
                        cps[half][0:HD + 1, :],
                        lhsT=v[t2][:, h * (HD + 1):(h + 1) * (HD + 1)],
                        rhs=Es[t2][:, half * 512:(half + 1) * 512],
                        start=(t2 == 0), stop=(t2 == TB - 1),
                    )
                for fn, a, kw in fills.get(t2, []):
                    fn(*a, **kw)

            if (n == NCH - 1) and (hp == CB - 1):
                # tail: m2's independent chunks first (they keep the PE busy
                # under the drain's reciprocal latency), then the final drain
                # on the freed scores banks with the hop on tail-idle ACT
                for j in range(CB - 1):
                    out_partial(2, n, j, bank=sc_half)
                drain_pair(cps, hp, n, sc_half, nc.scalar.copy)
                out_partial(0, n, 2)
                out_partial(1, n, 2)
            else:
                for half in range(2):
                    drain_bounce(cps[half], hp, half, n)

    # ---- tail: m3's independent chunks (cx slot freed by the drain), then
    # finish all four held accumulations; biases alternate DVE/ACT so neither
    # engine serializes the four stores; the last store is split so its first
    # DMA overlaps the second half's bias-add ----
    nl = NCH - 1
    for j in range(CB - 1):
        out_partial(3, nl, j, bank=cx_half)
    out_finish(0, nl, eng="v")
    out_finish(1, nl, eng="a", dma=nc.scalar)
    out_finish(2, nl, eng="v", dma=nc.gpsimd)
    out_finish(3, nl, split=2, eng="a", dma=nc.scalar)


_NC_CACHE = None


def _get_nc():
    global _NC_CACHE
    if _NC_CACHE is None:
        _NC_CACHE = build_nc()
    return _NC_CACHE


def _in_maps(x, Wq, bq, Wk, bk, Wv, bv, Wo, bo):
    x = np.ascontiguousarray(np.asarray(x, np.float32))
    wqT = np.asarray(Wq, np.float32).T   # [c_in, c_out]
    wkT = np.asarray(Wk, np.float32).T
    wvT = np.asarray(Wv, np.float32).T
    woT = np.asarray(Wo, np.float32).T

    base = {}
    # wqk{m}: [128, (j, q|k, 128)] — stationary slices for qk_group
    for m in range(CB):
        t = np.empty((128, 2 * C), np.float32)
        for j in range(CB):
            t[:, j * 256:j * 256 + 128] = wqT[j * 128:(j + 1) * 128,
                                              m * 128:(m + 1) * 128]
            t[:, j * 256 + 128:(j + 1) * 256] = wkT[j * 128:(j + 1) * 128,
                                                    m * 128:(m + 1) * 128]
        base[f"wqk{m}"] = t
    for h in range(2):
        base[f"wv{h}"] = np.ascontiguousarray(
            np.concatenate([wvT[(2 * h) * 128:(2 * h + 1) * 128, :],
                            wvT[(2 * h + 1) * 128:(2 * h + 2) * 128, :]], axis=1))
        base[f"wo{h}"] = np.ascontiguousarray(
            np.concatenate([woT[(2 * h) * 128:(2 * h + 1) * 128, :],
                            woT[(2 * h + 1) * 128:(2 * h + 2) * 128, :]], axis=1))
    mi = np.zeros((128, MISC_W), np.float32)
    mi[:, MC_BVBC:MC_BVBC + 512] = np.asarray(bv, np.float32)[None, :]
    for j in range(CB):
        mi[:, MC_BIAS + j] = np.asarray(bq, np.float32)[j * 128:(j + 1) * 128]
        mi[:, MC_BIAS + 4 + j] = np.asarray(bk, np.float32)[j * 128:(j + 1) * 128]
        mi[:, MC_BIAS + 8 + j] = np.asarray(bo, np.float32)[j * 128:(j + 1) * 128]
    mi[:, MC_ONES8:MC_ONES8 + 8] = 1.0
    mi[64, MC_SEL:MC_SEL + 64] = 1.0       # ones row for the drain broadcast
    base["misc"] = mi

    maps = []
    for b in range(B):
        xsT = np.ascontiguousarray(x[b].reshape(S, C).T)  # [C, S]
        m = dict(base)
        for j in range(CB):
            m[f"xst{j}"] = np.ascontiguousarray(xsT[j * 128:(j + 1) * 128, :])
        maps.append(m)
    return maps


def _run(trace=False, **inputs):
    nc = _get_nc()
    maps = _in_maps(**inputs)
    res = run_bass_kernel_spmd(nc, maps, core_ids=list(range(B)), trace=trace)
    out = np.stack(
        [np.asarray(res.results[b]["out"]).reshape(C, HH, WW) for b in range(B)]
    ).astype(np.float32)
    return out, res


def kernel(**inputs):
    out, _ = _run(trace=False, **inputs)
    return out


# revision 45
# speedup vs baseline: 1.0869x; 1.0210x over previous
"""Multi-head self-attention (B=8, E=512, heads=8, S=1024) on 8 trn2 cores.

Sharding: data-parallel over batch — core b computes batch element b end to
end (no collectives). Weights replicated, pre-transposed on host.

v2 design (cost-model-driven; see git history for the v1 layout):
  - xsT ([C, S]) is built on the HOST (the reference's reshape is a pure
    memory reinterpretation, so xsT = x[b].reshape(S, C).T in numpy). This
    removes all on-device PE transposes and their ACT/DVE copy traffic.
  - Loop order n (query half) OUTER, hp (head pair) INNER. The output
    projection for half n runs as PE filler inside half n+1's attention, so
    only the last half's projection sits in the tail.
  - All "filler" matmul groups (v-proj, q/k-proj, out-proj) accumulate in a
    dedicated 1-bank PSUM tag ("fg") so they never perturb the scores
    double-buffer; fillers are placed at fixed (n, hp, t2) slots chosen so
    every operand arrives just-in-time.
  - PSUM budget (8 banks): scores [128,1024] x2 (4) + ctx [65,512] x3 (3) +
    fg [128,512] x1 (1).
  - Warmup: gpsimd memset + 16 dummy matmuls finish the PE p-state ramp
    (0.65->2.4 GHz) before real work; a dummy exp preloads the ACT table.
  - Host packs wq+wk into per-m-slice tensors and biases/ones into one misc
    tensor; input DMAs are ordered by first use (HWDGE serializes issues).
  - Softmax denominators ride as a 65th stationary column of v (ones), so
    ctx PSUM row 64 accumulates them for free. Normalization: reciprocal on
    DVE, partition-broadcast via a DRAM bounce (mid-kernel, latency hidden)
    or via a K=1 PE matmul against a ones row (final drain, latency-critical).
"""

import numpy as np
import ml_dtypes
from contextlib import ExitStack

BF16 = ml_dtypes.bfloat16

import concourse.bass as bass
import concourse.mybir as mybir
import concourse.tile as tile
from concourse import bacc
from concourse.bass_utils import run_bass_kernel_spmd

B = 8
C = 512
HH = 32
WW = 32
S = HH * WW            # 1024
HEADS = 8
HD = C // HEADS        # 64
CB = C // 128          # 4 channel blocks
TB = S // 128          # 8 token blocks
CHUNK = 512            # fp32 moving-operand max
NCH = S // CHUNK       # 2
F32 = mybir.dt.float32
MM_DT = mybir.dt.float32r  # full-rate PE at N>=256 (warmups)
DT_X = mybir.dt.bfloat16   # streamed operands: full-rate PE, half DMA bytes

EXP = mybir.ActivationFunctionType.Exp
ADD = mybir.AluOpType.add
MULT = mybir.AluOpType.mult

# misc tensor column layout
MC_BVBC = 0          # [0:512)   bv broadcast along free dim
MC_BIAS = 512        # [512:524) bq(4), bk(4), bo(4) per-chunk scalars
MC_ONES8 = 524       # [524:532) ones for v's denominator columns
MC_SEL = 532         # [532:660) 2x128 selector (rows 64/65) for the drain
                     #           pair-broadcast matmul
MISC_W = 660


def build_nc(reps=1):
    nc = bacc.Bacc()
    xst_d = [nc.declare_dram_parameter(f"xst{j}", [128, S], DT_X, isOutput=False)
             for j in range(CB)]
    wqk_d = [nc.declare_dram_parameter(f"wqk{m}", [128, 2 * C], DT_X, isOutput=False)
             for m in range(CB)]
    wv_d = [nc.declare_dram_parameter(f"wv{h}", [128, 2 * C], DT_X, isOutput=False)
            for h in range(2)]
    wo_d = [nc.declare_dram_parameter(f"wo{h}", [128, 2 * C], DT_X, isOutput=False)
            for h in range(2)]
    misc_d = nc.declare_dram_parameter("misc", [128, MISC_W], F32, isOutput=False)
    out_d = nc.declare_dram_parameter("out", [C, S], F32, isOutput=True)

    with tile.TileContext(nc) as tc, ExitStack() as ctx:
        pools = _make_pools(ctx, tc)
        for _ in range(reps):
            _emit(pools, nc, xst_d, wqk_d, wv_d, wo_d, misc_d, out_d)
    nc.compile()
    return nc


def _make_pools(ctx, tc):
    return {
        "sb": ctx.enter_context(tc.tile_pool(name="sb", bufs=1)),
        "ps": ctx.enter_context(tc.tile_pool(name="ps", bufs=2, space="PSUM")),
        "ep": ctx.enter_context(tc.tile_pool(name="ep", bufs=6)),
        "np": ctx.enter_context(tc.tile_pool(name="npool", bufs=6)),
        "dr": ctx.enter_context(tc.tile_pool(name="drpool", bufs=4, space="DRAM")),
    }


def _emit(pools, nc, xst_d, wqk_d, wv_d, wo_d, misc_d, out_d):
    sb = pools["sb"]
    ps = pools["ps"]
    ep = pools["ep"]
    np_pool = pools["np"]
    dr_pool = pools["dr"]

    def sc_tile():
        return ps.tile([128, 1024], F32, tag="sc", bufs=2, name="sc")

    def cx_tile():
        return ps.tile([65, 512], F32, tag="cx", bufs=3, name="cx")

    def fg_tile():
        return ps.tile([128, 512], F32, tag="fg", bufs=1, name="fg")

    # ---- input DMAs, ordered by first use (HWDGE serializes issues) ----
    wqk = [sb.tile([128, 2 * C], DT_X, tag=f"wqk{m}", name=f"wqk{m}")
           for m in range(CB)]
    xsT = [sb.tile([128, S], DT_X, tag=f"xsT{j}", name=f"xsT{j}") for j in range(CB)]
    wv = [sb.tile([128, 2 * C], DT_X, tag=f"wv{h}", name=f"wv{h}") for h in range(2)]
    wo = [sb.tile([128, 2 * C], DT_X, tag=f"wo{h}", name=f"wo{h}") for h in range(2)]
    misc = sb.tile([128, MISC_W], F32, tag="misc", name="misc")

    nc.sync.dma_start(wqk[0], wqk_d[0][:, :])
    for j in range(CB):
        nc.sync.dma_start(xsT[j][:, 0:512], xst_d[j][:, 0:512])
    nc.sync.dma_start(misc, misc_d[:, :])
    nc.sync.dma_start(wv[0], wv_d[0][:, :])
    nc.sync.dma_start(wv[1], wv_d[1][:, :])
    for j in range(CB):
        nc.sync.dma_start(xsT[j][:, 512:1024], xst_d[j][:, 512:1024])
    for m in range(1, CB):
        nc.sync.dma_start(wqk[m], wqk_d[m][:, :])
    nc.sync.dma_start(wo[0], wo_d[0][:, :])
    nc.sync.dma_start(wo[1], wo_d[1][:, :])

    def w_slice(kind, j, m):
        # stationary [c_in 128, c_out 128] for projection matmuls
        if kind == "q":
            return wqk[m][:, j * 256:j * 256 + 128]
        if kind == "k":
            return wqk[m][:, j * 256 + 128:(j + 1) * 256]
        if kind == "v":
            return wv[j // 2][:, (j % 2) * 512:(j % 2) * 512 + 512]  # moving, 512 wide
        if kind == "o":
            return wo[j // 2][:, (j % 2) * 512 + m * 128:(j % 2) * 512 + (m + 1) * 128]
        raise KeyError(kind)

    def bias_ap(name, m):
        off = {"bq": 0, "bk": 4, "bo": 8}[name]
        return misc[:, MC_BIAS + off + m:MC_BIAS + off + m + 1]

    # ---- warmup: finish PE p-state ramp + preload the Exp ACT table ----
    wt = sb.tile([128, 512], F32, tag="wt", name="wt")
    nc.gpsimd.memset(wt[:, :], 0.0)
    wte = sb.tile([128, 8], F32, tag="wte", name="wte")
    nc.scalar.activation(wte, wt[:, 0:8], EXP, scale=0.125)

    def warm_mm():
        # dependency-free matmul: finishes the PE p-state ramp / bridges
        # DMA-arrival gaps so pe_busy_start never resets (result unread)
        pt = sc_tile()
        nc.tensor.matmul(pt[:, 0:512], lhsT=wt[:, 0:128].bitcast(MM_DT),
                         rhs=wt[:, 0:512].bitcast(MM_DT),
                         start=True, stop=True)

    for i in range(8):
        warm_mm()

    # ---- projection groups ----
    qT = [sb.tile([128, S], DT_X, tag=f"qT{m}", name=f"qT{m}") for m in range(CB)]
    kT = [sb.tile([128, S], DT_X, tag=f"kT{m}", name=f"kT{m}") for m in range(CB)]
    v = [sb.tile([128, HEADS * (HD + 1)], DT_X, tag=f"v{i}", name=f"v{i}")
         for i in range(TB)]
    zT = [sb.tile([128, S], DT_X, tag=f"zT{hp}", name=f"zT{hp}") for hp in range(CB)]
    outT = [sb.tile([128, S], F32, tag=f"outT{m}", name=f"outT{m}") for m in range(CB)]

    def qk_group(kind, m, n, bank=None):
        # qT/kT[m][:, n-half] = W[:, m-slice].T @ xsT[:, n-half] + bias
        dest = qT if kind == "q" else kT
        pt = bank() if bank else fg_tile()
        for j in range(CB):
            nc.tensor.matmul(
                pt[:, 0:512],
                lhsT=w_slice(kind, j, m),
                rhs=xsT[j][:, n * CHUNK:(n + 1) * CHUNK],
                start=(j == 0), stop=(j == CB - 1),
            )
        nc.vector.tensor_scalar_add(
            dest[m][:, n * CHUNK:(n + 1) * CHUNK], pt[:, 0:512],
            bias_ap("bq" if kind == "q" else "bk", m),
        )

    def v_group(i, bank=None):
        # v[i] token-major [128, 8*65]: head h dims at h*65..h*65+63, ones col
        # at h*65+64 (softmax denominator rides the ctx matmul).
        pt = bank() if bank else fg_tile()
        for j in range(CB):
            nc.tensor.matmul(
                pt[:, 0:512],
                lhsT=xsT[j][:, i * 128:(i + 1) * 128],
                rhs=w_slice("v", j, 0),
                start=(j == 0), stop=(j == CB - 1),
            )
        v3 = v[i].rearrange("p (h d) -> p h d", d=HD + 1)
        nc.vector.tensor_tensor(
            v3[:, :, 0:HD],
            pt[:, 0:512].rearrange("p (h d) -> p h d", d=HD),
            misc[:, MC_BVBC:MC_BVBC + 512].rearrange("p (h d) -> p h d", d=HD),
            ADD,
        )
        nc.vector.tensor_copy(v3[:, :, HD], misc[:, MC_ONES8:MC_ONES8 + 8])

    held = {}

    def out_mm(pt, m, n, j):
        nc.tensor.matmul(
            pt[:, 0:512],
            lhsT=w_slice("o", j, m),
            rhs=zT[j][:, n * CHUNK:(n + 1) * CHUNK],
            start=(j == 0), stop=(j == CB - 1),
        )

    def out_emit(pt, m, n, split=1, eng="v", dma=None):
        # bias + store for a finished out-proj accumulation; eng="a" runs the
        # bias on the ACT engine and dma= picks the issuing sequencer (the
        # tail's stores otherwise serialize on SP's 650ns/DMA issue rate).
        # All biases are emitted before any store so a DMA issue never sits
        # between two bias ops on the same engine sequencer.
        w = 512 // split
        for s in range(split):
            lo, hi = s * w, (s + 1) * w
            dst = outT[m][:, n * CHUNK + lo:n * CHUNK + hi]
            if eng == "a":
                nc.scalar.activation(
                    dst, pt[:, lo:hi], mybir.ActivationFunctionType.Identity,
                    bias=bias_ap("bo", m),
                )
            else:
                nc.vector.tensor_scalar_add(dst, pt[:, lo:hi], bias_ap("bo", m))
        for s in range(split):
            lo, hi = s * w, (s + 1) * w
            (dma or nc.sync).dma_start(
                out_d[m * 128:(m + 1) * 128, n * CHUNK + lo:n * CHUNK + hi],
                outT[m][:, n * CHUNK + lo:n * CHUNK + hi],
            )

    def out_group(m, n):
        # outT[m][:, n-half] = Wo[m-slice].T @ zT[:, n-half] + bo, then DMA
        pt = fg_tile()
        for j in range(CB):
            out_mm(pt, m, n, j)
        out_emit(pt, m, n)

    def cx_half():
        return ps.tile([128, 512], F32, tag="cx", bufs=3, name="cx")

    def out_partial(m, n, j, bank=None):
        # incremental out-proj chunk into a held accumulation (tail prep)
        if (m, n) not in held:
            held[(m, n)] = (bank or fg_tile)()
        out_mm(held[(m, n)], m, n, j)

    def out_finish(m, n, split=1, eng="v", dma=None):
        pt = held.pop((m, n))
        out_mm(pt, m, n, CB - 1)
        out_emit(pt, m, n, split=split, eng=eng, dma=dma)

    def sc_half():
        return ps.tile([128, 512], F32, tag="sc", bufs=2, name="sc")

    # ---- upfront groups (operands arrive via the first DMAs); spread over
    # sc + fg banks so they don't serialize on one accumulator. v0-v3 run
    # while the PE is otherwise DMA-idle; k01 is deferred (needed at t2=4).
    # k00's matmuls are paced by the xsT chunk DMAs, so warm matmuls ride
    # between them to keep the p-state ramp alive. ----
    ptk = fg_tile()
    for j in range(CB):
        nc.tensor.matmul(
            ptk[:, 0:512], lhsT=w_slice("k", j, 0), rhs=xsT[j][:, 0:CHUNK],
            start=(j == 0), stop=(j == CB - 1),
        )
        warm_mm()
        warm_mm()
    nc.vector.tensor_scalar_add(kT[0][:, 0:CHUNK], ptk[:, 0:512],
                                bias_ap("bk", 0))
    qk_group("q", 0, 0, bank=sc_half)   # sc slot A

    # filler schedule: (n, hp) -> {t2: thunk}; chosen so every group lands
    # just before its first consumer, consecutive fg users sit >= 1 iteration
    # apart (the fg matmul+bias round-trip is ~1us), and the tail carries no
    # q/k/v work.
    filler = {}

    def put(n, hp, t2, fn, *a, **k):
        filler.setdefault((n, hp), {}).setdefault(t2, []).append((fn, a, k))

    put(0, 0, 0, qk_group, "k", 0, 1)
    put(0, 0, 1, v_group, 4)
    put(0, 0, 2, v_group, 5)
    put(0, 0, 3, qk_group, "k", 1, 0)
    put(0, 0, 4, v_group, 6)
    put(0, 0, 5, qk_group, "q", 1, 0)
    put(0, 0, 6, v_group, 7)
    put(0, 0, 7, qk_group, "k", 1, 1)
    for mm in range(2, CB):
        put(0, mm - 1, 1, qk_group, "k", mm, 0)
        put(0, mm - 1, 3, qk_group, "q", mm, 0)
        put(0, mm - 1, 5, qk_group, "k", mm, 1)
    put(0, 3, 1, qk_group, "q", 0, 1)
    put(0, 3, 3, qk_group, "q", 1, 1)
    put(0, 3, 5, qk_group, "q", 2, 1)
    put(1, 0, 1, qk_group, "q", 3, 1)
    put(1, 1, 1, out_group, 0, 0)
    put(1, 1, 3, out_group, 1, 0)
    put(1, 1, 5, out_group, 2, 0)
    put(1, 2, 1, out_group, 3, 0)
    # tail prep: accumulate out(m=0/1, n=1) over already-drained zT chunks.
    # m0 in fg (free of drain(1,2)'s broadcast by t2=3); m1 in the cx slot
    # vacated by drain(1,2)'s accumulators. The j=2 chunks are saved for the
    # tail itself (PE-warming filler inside the final drain's bubble).
    put(1, 3, 3, out_partial, 0, 1, 0)
    put(1, 3, 4, out_partial, 1, 1, 0, bank=cx_half)
    put(1, 3, 5, out_partial, 0, 1, 1)
    put(1, 3, 5, out_partial, 1, 1, 1)

    def drain_bounce(cp, hp, half, n):
        # Mid-kernel normalization: 1/denominator at psum row 64, partition
        # broadcast via a DRAM bounce (no PE cost; ~6us latency hidden by the
        # cx rotation), multiply into zT.
        rs = np_pool.tile([65, 512], F32, tag="rs", name="rs")
        nc.vector.reciprocal(rs[64:65, :], cp[64:65, :])
        r_dram = dr_pool.tile([1, 512], F32, tag="r_dram", name="r_dram")
        nc.sync.dma_start(r_dram, rs[64:65, :])
        rb = np_pool.tile([64, 512], F32, tag="rb", name="rb")
        nc.sync.dma_start(rb, r_dram[0:1, :].partition_broadcast(64))
        nc.vector.tensor_tensor(
            zT[hp][half * 64:(half + 1) * 64, n * CHUNK:(n + 1) * CHUNK],
            cp[0:64, :], rb, MULT,
        )

    def drain_pair(cps, hp, n, bank, hop):
        # Normalize both ctx accumulators of a head pair: reciprocals of the
        # two denominator rows (psum row 64 of each), ONE K=2 matmul against
        # a 2x128 selector broadcasts recipA to partitions 0-63 and recipB to
        # 64-127, one PSUM->SBUF hop (DVE can't read two PSUM operands), two
        # multiplies into zT. Single PE instruction -> no boundary PE stall;
        # ~2.5us total so the cx rotation (bufs=3) never blocks.
        rs = [np_pool.tile([65, 512], F32, tag="rs", name="rs") for _ in range(2)]
        for half in range(2):
            nc.vector.reciprocal(rs[half][64:65, :], cps[half][64:65, :])
        rb = bank()
        for half in range(2):
            # K=1 broadcast matmuls into the two column-tiles of one bank:
            # back-to-back on PE, single PSUM->SBUF hop afterwards
            nc.tensor.matmul(
                rb[half * 64:(half + 1) * 64, 0:512],
                lhsT=misc[64:65, MC_SEL:MC_SEL + 64],
                rhs=rs[half][64:65, :],
                start=True, stop=True, tile_position=(64, half * 64),
            )
        rbs = np_pool.tile([128, 512], F32, tag="rbs", name="rbs")
        hop(rbs, rb[:, 0:512])
        for half in range(2):
            nc.vector.tensor_tensor(
                zT[hp][half * 64:(half + 1) * 64, n * CHUNK:(n + 1) * CHUNK],
                cps[half][0:64, :], rbs[half * 64:(half + 1) * 64, :], MULT,
            )

    # ---- attention: n outer, hp inner; scores/exp emitted one t2 ahead so
    # a ctx matmul waiting on exp never blocks the scores pipeline ----
    for n in range(NCH):
        for hp in range(CB):
            qh, kh = qT[hp], kT[hp]
            fills = filler.get((n, hp), {})
            cps = [cx_tile(), cx_tile()]   # head A, head B
            Es = [None] * TB

            def emit_se(t2):
                sc = sc_tile()
                nc.tensor.matmul(
                    sc[:, 0:512],
                    lhsT=kh[0:64, t2 * 128:(t2 + 1) * 128],
                    rhs=qh[0:64, n * CHUNK:(n + 1) * CHUNK],
                    start=True, stop=True,
                    tile_position=(0, 0),
                )
                nc.tensor.matmul(
                    sc[:, 512:1024],
                    lhsT=kh[64:128, t2 * 128:(t2 + 1) * 128],
                    rhs=qh[64:128, n * CHUNK:(n + 1) * CHUNK],
                    start=True, stop=True,
                    tile_position=(64, 0),
                )
                Es[t2] = ep.tile([128, 1024], DT_X, tag="E", name="E")
                nc.scalar.activation(Es[t2], sc, EXP, scale=1.0 / np.sqrt(HD))

            # two-ahead: se(t2+2) is emitted before ctx(t2) so the ACT
            # pipeline never waits on a ctx-blocked PE (the dependency cycle
            # exp(t2)->ctx(t2)->scores(t2+2)->exp(t2+2) would be 1.11us,
            # longer than one 1.04us exp)
            emit_se(0)
            emit_se(1)
            if n == 0 and hp == 0:
                # v0-v3 land in the startup DMA window; two lanes (fg + the
                # one spare cx slot) so they don't serialize on one bank
                v_group(0)
                v_group(1, bank=cx_half)
                v_group(2)
                v_group(3)
            for t2 in range(TB):
                if t2 + 2 < TB:
                    emit_se(t2 + 2)
                for half in range(2):
                    h = 2 * hp + half
                    nc.tensor.matmul(
                        cps[half][0:HD + 1, :],
                        lhsT=v[t2][:, h * (HD + 1):(h + 1) * (HD + 1)],
                        rhs=Es[t2][:, half * 512:(half + 1) * 512],
                        start=(t2 == 0), stop=(t2 == TB - 1),
                    )
                for fn, a, kw in fills.get(t2, []):
                    fn(*a, **kw)

            if (n == NCH - 1) and (hp == CB - 1):
                # tail: m2's independent chunks first (they keep the PE busy
                # under the drain's reciprocal latency), then the final drain
                # on the freed scores banks with the hop on tail-idle ACT
                for j in range(CB - 1):
                    out_partial(2, n, j, bank=sc_half)
                drain_pair(cps, hp, n, sc_half, nc.scalar.copy)
                out_partial(0, n, 2)
                out_partial(1, n, 2)
            else:
                for half in range(2):
                    drain_bounce(cps[half], hp, half, n)

    # ---- tail: m3's independent chunks (cx slot freed by the drain), then
    # finish all four held accumulations; biases alternate DVE/ACT so neither
    # engine serializes the four stores; the last store is split so its first
    # DMA overlaps the second half's bias-add ----
    nl = NCH - 1
    for j in range(CB - 1):
        out_partial(3, nl, j, bank=cx_half)
    out_finish(0, nl, eng="v")
    out_finish(1, nl, eng="a", dma=nc.scalar)
    out_finish(2, nl, eng="v")
    out_finish(3, nl, split=2, eng="a", dma=nc.scalar)


_NC_CACHE = None


def _get_nc():
    global _NC_CACHE
    if _NC_CACHE is None:
        _NC_CACHE = build_nc()
    return _NC_CACHE


def _in_maps(x, Wq, bq, Wk, bk, Wv, bv, Wo, bo):
    x = np.ascontiguousarray(np.asarray(x, np.float32))
    wqT = np.asarray(Wq, np.float32).T.astype(BF16)   # [c_in, c_out]
    wkT = np.asarray(Wk, np.float32).T.astype(BF16)
    wvT = np.asarray(Wv, np.float32).T.astype(BF16)
    woT = np.asarray(Wo, np.float32).T.astype(BF16)

    base = {}
    # wqk{m}: [128, (j, q|k, 128)] — stationary slices for qk_group
    for m in range(CB):
        t = np.empty((128, 2 * C), BF16)
        for j in range(CB):
            t[:, j * 256:j * 256 + 128] = wqT[j * 128:(j + 1) * 128,
                                              m * 128:(m + 1) * 128]
            t[:, j * 256 + 128:(j + 1) * 256] = wkT[j * 128:(j + 1) * 128,
                                                    m * 128:(m + 1) * 128]
        base[f"wqk{m}"] = t
    for h in range(2):
        base[f"wv{h}"] = np.ascontiguousarray(
            np.concatenate([wvT[(2 * h) * 128:(2 * h + 1) * 128, :],
                            wvT[(2 * h + 1) * 128:(2 * h + 2) * 128, :]], axis=1))
        base[f"wo{h}"] = np.ascontiguousarray(
            np.concatenate([woT[(2 * h) * 128:(2 * h + 1) * 128, :],
                            woT[(2 * h + 1) * 128:(2 * h + 2) * 128, :]], axis=1))
    mi = np.zeros((128, MISC_W), np.float32)
    mi[:, MC_BVBC:MC_BVBC + 512] = np.asarray(bv, np.float32)[None, :]
    for j in range(CB):
        mi[:, MC_BIAS + j] = np.asarray(bq, np.float32)[j * 128:(j + 1) * 128]
        mi[:, MC_BIAS + 4 + j] = np.asarray(bk, np.float32)[j * 128:(j + 1) * 128]
        mi[:, MC_BIAS + 8 + j] = np.asarray(bo, np.float32)[j * 128:(j + 1) * 128]
    mi[:, MC_ONES8:MC_ONES8 + 8] = 1.0
    mi[64, MC_SEL:MC_SEL + 64] = 1.0       # ones row for the drain broadcast
    base["misc"] = mi

    maps = []
    for b in range(B):
        xsT = np.ascontiguousarray(x[b].reshape(S, C).T).astype(BF16)  # [C, S]
        m = dict(base)
        for j in range(CB):
            m[f"xst{j}"] = np.ascontiguousarray(xsT[j * 128:(j + 1) * 128, :])
        maps.append(m)
    return maps


def _run(trace=False, **inputs):
    nc = _get_nc()
    maps = _in_maps(**inputs)
    res = run_bass_kernel_spmd(nc, maps, core_ids=list(range(B)), trace=trace)
    out = np.stack(
        [np.asarray(res.results[b]["out"]).reshape(C, HH, WW) for b in range(B)]
    ).astype(np.float32)
    return out, res


def kernel(**inputs):
    out, _ = _run(trace=False, **inputs)
    return out
